# revision 22
# baseline (speedup 1.0000x reference)
"""Self-contained 8-core Trainium2 kernel for the 6-layer dense transformer.

Sharding: batch pairs with head-split attention. Core c owns batch b=c//2 and
sequence half hf=c%2 (512 tokens) for the residual stream, LayerNorms, FFN and
LM head. Attention for batch b is split by heads across the pair: core 2b
computes heads 0-7, core 2b+1 heads 8-15, each over all 1024 tokens, so K/V
never cross cores. The only collectives are pairwise: an AllGather of the LN1
output (so both cores see all 1024 tokens) and a ReduceScatter of the Wo
partial products (each core receives the summed attention output for its own
tokens). The LM head is local: every core holds the full bf16 W_out.

Activations live feature-major (x^T: [D, tokens]); residual stays f32,
LN statistics f32r, everything else (projections, attention, FFN, LM head)
bf16 with f32 PSUM accumulation. Weights are pre-tiled on the host so every
weight DMA is partition-contiguous. Softmax denominators are inverted via
exp(-ln(d)) on the scalar engine (DVE reciprocal is ~25x slower per element).
The embedding lookup runs on the host; the device loads x0^T directly.
"""

import numpy as np

B, T, D, H, HS, L, DFF, V = 4, 1024, 1024, 16, 64, 6, 4096, 32000
NCORES = 8
TC = 512            # tokens owned per core
TB = 1024           # tokens per batch (attention span)
P = 128
VB = V // P         # 250 vocab row-blocks
EPS = 1e-5

_CACHE = {}
TRACE = False
LAST_RESULTS = None


def _build():
    import concourse.bacc as bacc
    import concourse.tile as tile
    import concourse.mybir as mybir
    from contextlib import ExitStack

    f32 = mybir.dt.float32
    f32r = mybir.dt.float32r
    bf16 = mybir.dt.bfloat16
    AF = mybir.ActivationFunctionType
    ALU = mybir.AluOpType

    nc = bacc.Bacc(None, target_bir_lowering=False, debug=False,
                   num_devices=NCORES)

    # ---- parameters (host pre-tiled layouts; wq/wk/wv/wo/lnpp per-core) ----
    x0T = nc.declare_dram_parameter("x0T", [D, TC], f32, isOutput=False)
    wqp = nc.declare_dram_parameter("wqp", [L, P, 8, 4, P], bf16, isOutput=False)
    wkp = nc.declare_dram_parameter("wkp", [L, P, 8, 4, P], bf16, isOutput=False)
    wvp = nc.declare_dram_parameter("wvp", [L, P, 8, 512], bf16, isOutput=False)
    wop = nc.declare_dram_parameter("wop", [L, P, 4, 8, P], bf16, isOutput=False)
    w1p = nc.declare_dram_parameter("w1p", [L, 32, P, 8, P], bf16, isOutput=False)
    w2p = nc.declare_dram_parameter("w2p", [L, 4, 8, P, 8, P], bf16, isOutput=False)
    woutp = nc.declare_dram_parameter("woutp", [VB, P, 8, P], bf16, isOutput=False)
    boutp = nc.declare_dram_parameter("boutp", [P, VB], f32, isOutput=False)
    lnpp = nc.declare_dram_parameter("lnpp", [L, P, 48], f32, isOutput=False)
    b1p = nc.declare_dram_parameter("b1p", [L, P, 32], f32, isOutput=False)
    lnfp = nc.declare_dram_parameter("lnfp", [P, 16], f32, isOutput=False)
    cmask = nc.declare_dram_parameter("cmask", [4, P, TC], bf16, isOutput=False)
    out = nc.declare_dram_parameter("out", [V, TC], f32, isOutput=True)

    RG2 = [[0, 1], [2, 3], [4, 5], [6, 7]]

    with tile.TileContext(nc) as tc:
        outer = ExitStack()
        singles = outer.enter_context(tc.tile_pool(name="singles", bufs=1))
        dramp = outer.enter_context(tc.tile_pool(name="dramp", bufs=1, space="DRAM"))

        # ---- internal DRAM (per layer and split in halves so each collective
        # can start as soon as its half of the data is staged) ----
        HD = D // 2
        xln_locs = [[dramp.tile([HD, TC], bf16, name=f"xln_loc_{i}_{h}")
                     for h in range(2)] for i in range(L)]
        xln_pairs = [[dramp.tile([D, TC], bf16, name=f"xln_pair_{i}_{h}")
                      for h in range(2)] for i in range(L)]
        part_locs = [[dramp.tile([D, TC], bf16, name=f"part_loc_{i}_{h}")
                      for h in range(2)] for i in range(L)]
        attn_rss = [[dramp.tile([HD, TC], bf16, name=f"attn_rs_{i}_{h}")
                     for h in range(2)] for i in range(L)]

        # constants
        ones_f = singles.tile([P, 144], f32, name="ones_f")
        nc.vector.memset(ones_f[:], 1.0)
        ones_r = singles.tile([P, 144], f32r, name="ones_r")
        nc.vector.tensor_copy(out=ones_r[:], in_=ones_f[:])
        eps_c = singles.tile([P, 1], f32, name="eps_c")
        nc.vector.memset(eps_c[:], EPS)
        ones_b = singles.tile([P, P], bf16, name="ones_b")
        nc.vector.memset(ones_b[:], 1.0)
        bout_t = singles.tile([P, VB], f32, name="bout_t")
        nc.sync.dma_start(out=bout_t[:], in_=boutp[:])
        cm_t = []
        for j in range(4):
            mt = singles.tile([P, TC], bf16, name=f"cm_{j}")
            nc.sync.dma_start(out=mt[:], in_=cmask[j])
            cm_t.append(mt)

        est = ExitStack()
        lp = est.enter_context(tc.tile_pool(name="lp", bufs=1))      # xr/xln tiles
        kqp = est.enter_context(tc.tile_pool(name="kqp", bufs=1))    # K/Q [P,1024]
        vtsp = est.enter_context(tc.tile_pool(name="vtsp", bufs=1))  # V 65-strided
        osp = est.enter_context(tc.tile_pool(name="osp", bufs=1))    # o [P,1024]
        big = est.enter_context(tc.tile_pool(name="big", bufs=1))    # h1/y2 slots
        wL = est.enter_context(tc.tile_pool(name="wL", bufs=1))      # layer weights
        wA = est.enter_context(tc.tile_pool(name="wA", bufs=3))      # ffn stream
        sp = est.enter_context(tc.tile_pool(name="sp", bufs=2))      # stream tiles
        esp = est.enter_context(tc.tile_pool(name="esp", bufs=3))    # exp(scores)
        otp = est.enter_context(tc.tile_pool(name="otp", bufs=2))    # denom tmp
        stp = est.enter_context(tc.tile_pool(name="stp", bufs=1))    # LN stats [1,*]

        ps_mm = est.enter_context(tc.tile_pool(name="ps_mm", bufs=2, space="PSUM"))
        ps_o = est.enter_context(tc.tile_pool(name="ps_o", bufs=1, space="PSUM"))
        ps_st = est.enter_context(tc.tile_pool(name="ps_st", bufs=1, space="PSUM"))
        ps_bc = est.enter_context(tc.tile_pool(name="ps_bc", bufs=1, space="PSUM"))

        def mmtile():
            return ps_mm.tile([P, TC], f32, name="mm", tag="mm")

        xr = [lp.tile([P, TC], f32, name=f"xr_{j}", tag=f"xr_{j}") for j in range(8)]

        def own_tiles(dtype=bf16):
            return [lp.tile([P, TC], dtype, name=f"xln_{j}", tag=f"xln_{j}")
                    for j in range(8)]

        # persistent attention tiles. kz: one tile per head, the head's 64 K
        # rows in its parity half and ZEROS in the other half, so the score
        # matmul runs with a full 128x128 stationary (full PE rate); the rhs
        # reads the packed Q tile whose other half contributes 0 via the zeros.
        kz = [kqp.tile([P, TB], bf16, name=f"kz_{i}", tag=f"kz_{i}")
              for i in range(8)]
        for i in range(8):
            nc.vector.memset(kz[i][:], 0.0)
        kq_q = [kqp.tile([P, TB], bf16, name=f"kq_{i}", tag=f"kq_{i}")
                for i in range(4)]
        # 584 wide so a 128-col stationary window starting at 65*hh always fits
        vts = [vtsp.tile([P, 584], bf16, name=f"vts_{i}", tag=f"vts_{i}")
               for i in range(8)]
        for i in range(8):
            nc.vector.memset(vts[i][:], 1.0)   # ones cols (64 of each 65) persist
        o_sb = [osp.tile([P, TB], bf16, name=f"osb_{i}", tag=f"osb_{i}")
                for i in range(4)]

        def big_tile(i, name, dtype=bf16):
            return big.tile([P, TC], dtype, name=name, tag=f"big_{i}")

        # ---- embedding: host-precomputed x0T, straight loads ----
        for j in range(8):
            nc.sync.dma_start(out=xr[j][:], in_=x0T[j * P:(j + 1) * P, :])

        def layer_norm(g_t, gcol, b_t, bcol, out_tiles):
            """xr (f32) -> out_tiles; feature-major LN over partitions.

            Stats accumulate through a full 128x128 ones stationary, which
            lands them already broadcast along partitions at full PE rate.
            """
            sum_bc = ps_bc.tile([P, TC], f32, name="sum_bc", tag="bc_a")
            sumsq_bc = ps_bc.tile([P, TC], f32, name="sumsq_bc", tag="bc_c")
            for j in range(8):
                xc = sp.tile([P, TC], bf16, name="ln_xc", tag="ln_xc")
                nc.scalar.activation(out=xc[:], in_=xr[j][:], func=AF.Copy)
                sq = sp.tile([P, TC], bf16, name="ln_sq", tag="ln_sq")
                nc.scalar.activation(out=sq[:], in_=xr[j][:], func=AF.Square)
                nc.tensor.matmul(out=sum_bc[:], lhsT=ones_b[:], rhs=xc[:],
                                 start=(j == 0), stop=(j == 7))
                nc.tensor.matmul(out=sumsq_bc[:], lhsT=ones_b[:], rhs=sq[:],
                                 start=(j == 0), stop=(j == 7))
            nmean = sp.tile([P, TC], f32, name="ln_mb", tag="ln_mb")
            nc.scalar.activation(out=nmean[:], in_=sum_bc[:], func=AF.Copy,
                                 scale=-1.0 / D)
            msq = sp.tile([P, TC], f32, name="ln_msq", tag="ln_msq")
            nc.scalar.activation(out=msq[:], in_=sumsq_bc[:], func=AF.Copy,
                                 scale=1.0 / D)
            m2 = sp.tile([P, TC], f32, name="ln_m2b", tag="ln_m2b")
            nc.vector.tensor_mul(out=m2[:], in0=nmean[:], in1=nmean[:])
            nc.vector.tensor_tensor(out=msq[:], in0=msq[:], in1=m2[:],
                                    op=ALU.subtract)
            std = sp.tile([P, TC], f32, name="ln_sb", tag="ln_sb")
            nc.scalar.activation(out=std[:], in_=msq[:], func=AF.Sqrt,
                                 bias=eps_c[:], scale=1.0)
            rstd_bc = sp.tile([P, TC], f32, name="ln_rb", tag="ln_rb")
            nc.vector.reciprocal(out=rstd_bc[:], in_=std[:])
            for j in range(8):
                t1 = sp.tile([P, TC], f32, name="ln_t1", tag="ln_t1")
                nc.vector.tensor_add(out=t1[:], in0=xr[j][:], in1=nmean[:])
                nc.vector.tensor_mul(out=t1[:], in0=t1[:], in1=rstd_bc[:])
                nc.vector.tensor_scalar(
                    out=out_tiles[j][:], in0=t1[:],
                    scalar1=g_t[:, gcol + j:gcol + j + 1],
                    scalar2=b_t[:, bcol + j:bcol + j + 1],
                    op0=ALU.mult, op1=ALU.add)

        # ================= layers =================
        for l in range(L):
            lnp = sp.tile([P, 48], f32, name="lnp", tag="lnp")
            nc.sync.dma_start(out=lnp[:], in_=lnpp[l])
            b1f_t = sp.tile([P, 32], f32, name="b1f_t", tag="b1f_t")
            nc.sync.dma_start(out=b1f_t[:], in_=b1p[l])

            # ---- LN1 on own tokens -> store -> two pipelined pair AllGathers ----
            xln1 = own_tiles()
            layer_norm(lnp, 0, lnp, 8, xln1)
            for h in range(2):
                for j4 in range(4):
                    nc.sync.dma_start(
                        out=xln_locs[l][h][j4 * P:(j4 + 1) * P, :],
                        in_=xln1[4 * h + j4][:])
                nc.gpsimd.collective_compute(
                    "AllGather", ALU.bypass, replica_groups=RG2,
                    ins=[xln_locs[l][h].opt()], outs=[xln_pairs[l][h].opt()])

            # reload the gathered 1024-token activations (feature-major)
            xf_full = [lp.tile([P, TB], bf16, name=f"xf_{j}", tag=f"xf_{j}")
                       for j in range(8)]
            for k in range(8):
                h, k4 = k // 4, k % 4
                nc.sync.dma_start(out=xf_full[k][:, 0:TC],
                                  in_=xln_pairs[l][h][k4 * P:(k4 + 1) * P, :])
                nc.sync.dma_start(
                    out=xf_full[k][:, TC:TB],
                    in_=xln_pairs[l][h][HD + k4 * P:HD + (k4 + 1) * P, :])

            # ---- layer weight tiles (one contiguous DMA each) ----
            wk_t = wL.tile([P, 8, 4, P], bf16, name="wk_t", tag="wk")
            nc.sync.dma_start(out=wk_t[:], in_=wkp[l])
            wq_t = wL.tile([P, 8, 4, P], bf16, name="wq_t", tag="wq")
            nc.sync.dma_start(out=wq_t[:], in_=wqp[l])
            wv_t = wL.tile([P, 8, 512], bf16, name="wv_t", tag="wv")
            nc.sync.dma_start(out=wv_t[:], in_=wvp[l])
            wo_t = wL.tile([P, 4, 8, P], bf16, name="wo_t", tag="wo")
            nc.sync.dma_start(out=wo_t[:], in_=wop[l])

            # ---- K and Q projections: my 8 heads x 1024 tokens ----
            for oc in range(4):
                for th in range(2):
                    ps = mmtile()
                    for k in range(8):
                        nc.tensor.matmul(
                            out=ps[:], lhsT=wk_t[:, k, oc, :],
                            rhs=xf_full[k][:, th * TC:(th + 1) * TC],
                            start=(k == 0), stop=(k == 7))
                    # split row halves into the two heads' zero-padded K tiles
                    for ph in range(2):
                        nc.scalar.activation(
                            out=kz[2 * oc + ph][64 * ph:64 * ph + 64,
                                                th * TC:(th + 1) * TC],
                            in_=ps[64 * ph:64 * ph + 64, :], func=AF.Copy)
            for oc in range(4):
                for th in range(2):
                    ps = mmtile()
                    for k in range(8):
                        nc.tensor.matmul(
                            out=ps[:], lhsT=wq_t[:, k, oc, :],
                            rhs=xf_full[k][:, th * TC:(th + 1) * TC],
                            start=(k == 0), stop=(k == 7))
                    nc.scalar.activation(
                        out=kq_q[oc][:, th * TC:(th + 1) * TC], in_=ps[:],
                        func=AF.Copy)

            # ---- V projection: [tokens, head-dims], 65-strided with ones ----
            for tcn in range(8):
                ps = mmtile()
                for k in range(8):
                    nc.tensor.matmul(
                        out=ps[:], lhsT=xf_full[k][:, tcn * P:(tcn + 1) * P],
                        rhs=wv_t[:, k, :], start=(k == 0), stop=(k == 7))
                vsb = sp.tile([P, 512], bf16, name="vsb", tag="vsb")
                nc.scalar.activation(out=vsb[:], in_=ps[:], func=AF.Copy)
                nc.sync.dma_start(
                    out=vts[tcn][:, 0:520].rearrange("p (h c) -> p h c",
                                                     c=65)[:, :, 0:64],
                    in_=vsb[:].rearrange("p (h c) -> p h c", c=64))

            # ---- attention: 8 local heads, interleaved in pairs so one head's
            # matmuls fill the other's softmax bubbles ----
            for hp in range(4):
                qt = kq_q[hp]
                opst = [None, None]
                for sl in range(2):       # slot: even head -> ps_o, odd -> ps_st
                    pso = ps_o if sl == 0 else ps_st
                    tga, tgb = (("ops_lo", "ops_hi") if sl == 0
                                else ("st_a", "st_b"))
                    opst[sl] = (pso.tile([P, TC], f32, name="ops_lo", tag=tga),
                                pso.tile([P, TC], f32, name="ops_hi", tag=tgb))
                for j in range(8):
                    for sl in range(2):
                        hh = 2 * hp + sl
                        ops_lo, ops_hi = opst[sl]
                        vwin = vts[j][:, 65 * hh:65 * hh + P]
                        es_hi = esp.tile([P, TC], bf16, name="es_h",
                                         tag=f"es_h{sl}")
                        if j < 4:
                            # lo half: q columns [128j, 512), diagonal-masked
                            q0 = P * j
                            sps = mmtile()
                            nc.tensor.matmul(
                                out=sps[:, q0:TC],
                                lhsT=kz[hh][:, j * P:(j + 1) * P],
                                rhs=qt[:, q0:TC],
                                start=True, stop=True)
                            es_lo = esp.tile([P, TC], bf16, name="es_l",
                                             tag=f"es_l{sl}")
                            nc.scalar.activation(out=es_lo[:, q0:TC],
                                                 in_=sps[:, q0:TC],
                                                 func=AF.Exp, scale=HS ** -0.5)
                            nc.vector.tensor_mul(out=es_lo[:, q0:TC],
                                                 in0=es_lo[:, q0:TC],
                                                 in1=cm_t[j][:, q0:TC])
                            nc.tensor.matmul(
                                out=ops_lo[:, q0:TC], lhsT=vwin,
                                rhs=es_lo[:, q0:TC],
                                start=(j == 0), stop=(j == 3))
                            # hi half: q columns [512, 1024), fully visible
                            sps2 = mmtile()
                            nc.tensor.matmul(
                                out=sps2[:],
                                lhsT=kz[hh][:, j * P:(j + 1) * P],
                                rhs=qt[:, TC:TB],
                                start=True, stop=True)
                            nc.scalar.activation(out=es_hi[:], in_=sps2[:],
                                                 func=AF.Exp, scale=HS ** -0.5)
                        else:
                            # hi half only: q columns [512+128(j-4), 1024)
                            q0 = P * (j - 4)
                            sps = mmtile()
                            nc.tensor.matmul(
                                out=sps[:, q0:TC],
                                lhsT=kz[hh][:, j * P:(j + 1) * P],
                                rhs=qt[:, TC + q0:TB],
                                start=True, stop=True)
                            nc.scalar.activation(out=es_hi[:, q0:TC],
                                                 in_=sps[:, q0:TC],
                                                 func=AF.Exp, scale=HS ** -0.5)
                            nc.vector.tensor_mul(out=es_hi[:, q0:TC],
                                                 in0=es_hi[:, q0:TC],
                                                 in1=cm_t[j - 4][:, q0:TC])
                            nc.tensor.matmul(
                                out=ops_hi[:, q0:TC], lhsT=vwin,
                                rhs=es_hi[:, q0:TC], start=False, stop=(j == 7))
                            continue
                        nc.tensor.matmul(
                            out=ops_hi[:], lhsT=vwin,
                            rhs=es_hi[:], start=(j == 0), stop=False)
                # normalize: o = ops[0:64] / ops[64] (ones row)
                for sl in range(2):
                    hh = 2 * hp + sl
                    base = 64 * sl
                    for ih, opsx in ((0, opst[sl][0]), (1, opst[sl][1])):
                        dsb = otp.tile([1, TC], f32r, name="dsb", tag="dsb")
                        with nc.allow_low_precision(reason="f32r softmax denom"):
                            nc.scalar.activation(out=dsb[:], in_=opsx[64:65, :],
                                                 func=AF.Copy)
                        bc = ps_bc.tile([P, TC], f32, name="bc_o", tag="bc_a")
                        nc.tensor.matmul(out=bc[:], lhsT=ones_r[0:1, 0:P],
                                         rhs=dsb[:], start=True, stop=True)
                        bcr = otp.tile([64, TC], bf16, name="bcr", tag="bcr")
                        with nc.allow_low_precision(reason="bf16 denom recip"):
                            nc.vector.reciprocal(out=bcr[:], in_=bc[0:64, :])
                        nc.vector.tensor_mul(
                            out=o_sb[hp][base:base + 64, ih * TC:(ih + 1) * TC],
                            in0=opsx[0:64, :], in1=bcr[:])

            # ---- Wo partials -> staging -> two pipelined pair ReduceScatters ----
            for h in range(2):
                for d4 in range(4):
                    dout = 4 * h + d4
                    for th in range(2):
                        ps = mmtile()
                        for kc in range(4):
                            nc.tensor.matmul(
                                out=ps[:], lhsT=wo_t[:, kc, dout, :],
                                rhs=o_sb[kc][:, th * TC:(th + 1) * TC],
                                start=(kc == 0), stop=(kc == 3))
                        psb = sp.tile([P, TC], bf16, name="psb", tag="psb")
                        nc.scalar.activation(out=psb[:], in_=ps[:],
                                             func=AF.Identity,
                                             bias=lnp[:, 32 + dout:33 + dout],
                                             scale=1.0)
                        nc.sync.dma_start(
                            out=part_locs[l][h][th * HD + d4 * P:
                                                th * HD + (d4 + 1) * P, :],
                            in_=psb[:])
                nc.gpsimd.collective_compute(
                    "ReduceScatter", ALU.add, replica_groups=RG2,
                    ins=[part_locs[l][h].opt()], outs=[attn_rss[l][h].opt()])

            # ---- residual add from the scattered attention output ----
            for dout in range(8):
                h, d4 = dout // 4, dout % 4
                ar = sp.tile([P, TC], bf16, name="ar", tag="ar")
                nc.sync.dma_start(out=ar[:],
                                  in_=attn_rss[l][h][d4 * P:(d4 + 1) * P, :])
                nc.vector.tensor_add(out=xr[dout][:], in0=xr[dout][:], in1=ar[:])

            # ---- FFN (own 512 tokens, bf16) ----
            xln2 = own_tiles()
            layer_norm(lnp, 16, lnp, 24, xln2)

            y2 = [big_tile(8 + d_, f"y2_{d_}", dtype=f32) for d_ in range(8)]
            for blk in range(4):
                h1 = [big_tile(c_, f"h1_{c_}") for c_ in range(8)]
                for ci in range(8):
                    hc = 8 * blk + ci
                    wt = wA.tile([P, 8, P], bf16, name="w1_t", tag="wA")
                    nc.sync.dma_start(out=wt[:], in_=w1p[l, hc])
                    ps = mmtile()
                    for k in range(8):
                        nc.tensor.matmul(out=ps[:], lhsT=wt[:, k, :], rhs=xln2[k][:],
                                         start=(k == 0), stop=(k == 7))
                    nc.scalar.activation(out=h1[ci][:], in_=ps[:], func=AF.Relu,
                                         bias=b1f_t[:, hc:hc + 1], scale=1.0)
                for dout in range(8):
                    wt = wA.tile([P, 8, P], bf16, name="w2_t", tag="wA")
                    nc.sync.dma_start(out=wt[:], in_=w2p[l, blk, dout])
                    ps = mmtile()
                    for c in range(8):
                        nc.tensor.matmul(out=ps[:], lhsT=wt[:, c, :], rhs=h1[c][:],
                                         start=(c == 0), stop=(c == 7))
                    if blk == 0:
                        nc.scalar.activation(out=y2[dout][:], in_=ps[:],
                                             func=AF.Identity,
                                             bias=lnp[:, 40 + dout:41 + dout],
                                             scale=1.0)
                    else:
                        nc.vector.tensor_add(out=y2[dout][:], in0=y2[dout][:],
                                             in1=ps[:])
            for dout in range(8):
                nc.vector.tensor_add(out=xr[dout][:], in0=xr[dout][:],
                                     in1=y2[dout][:])

        # ---- final LN (bf16 output for the LM head) ----
        lnf_t = sp.tile([P, 16], f32, name="lnf_t", tag="lnp")
        nc.sync.dma_start(out=lnf_t[:], in_=lnfp[:])
        xlnf = own_tiles()
        layer_norm(lnf_t, 0, lnf_t, 8, xlnf)

        est.close()

        # ================= LM head (local, full vocab) =================
        with tc.tile_pool(name="lmw", bufs=4) as lmw, \
             tc.tile_pool(name="lmo", bufs=3) as lmo, \
             tc.tile_pool(name="ps_lm", bufs=4, space="PSUM") as ps_lm:
            for vb in range(VB):
                wt = lmw.tile([P, 8, P], bf16, name="wout_t", tag="wout")
                nc.sync.dma_start(out=wt[:], in_=woutp[vb])
                ps = ps_lm.tile([P, TC], f32, name="lm_ps", tag="lm")
                for k in range(8):
                    nc.tensor.matmul(out=ps[:], lhsT=wt[:, k, :], rhs=xlnf[k][:],
                                     start=(k == 0), stop=(k == 7))
                osb = lmo.tile([P, TC], f32, name="osb", tag="osb")
                nc.scalar.activation(out=osb[:], in_=ps[:], func=AF.Identity,
                                     bias=bout_t[:, vb:vb + 1], scale=1.0)
                nc.sync.dma_start(out=out[vb * P:(vb + 1) * P, :], in_=osb[:])
        outer.close()

    nc.compile()
    return nc


def _prep_inputs(inputs):
    """Shard/reformat host inputs into 8 per-core input maps."""
    import ml_dtypes
    bf = ml_dtypes.bfloat16
    inp = {k: np.asarray(v) for k, v in inputs.items()}
    tok = inp['input_tokens'].astype(np.int64)          # [B, T]
    temb = np.asarray(inp['tok_emb'], dtype=np.float32)
    pe = np.asarray(inp['pos_emb'], dtype=np.float32)

    w1 = np.asarray(inp['W1'], np.float32)
    w2 = np.asarray(inp['W2'], np.float32)
    shared = {
        'w1p': np.ascontiguousarray(
            w1.reshape(L, 8, P, 32, P).transpose(0, 3, 2, 1, 4)).astype(bf),
        'w2p': np.ascontiguousarray(
            w2.reshape(L, 4, 8, P, 8, P).transpose(0, 1, 4, 3, 2, 5)).astype(bf),
        'woutp': np.ascontiguousarray(
            np.asarray(inp['W_out'], np.float32)
            .reshape(8, P, VB, P).transpose(2, 1, 0, 3)).astype(bf),
        'boutp': np.ascontiguousarray(
            np.asarray(inp['b_out'], np.float32).reshape(VB, P).T),
        'b1p': np.ascontiguousarray(
            np.asarray(inp['b1'], np.float32).reshape(L, 32, P).transpose(0, 2, 1)),
        'lnfp': np.ascontiguousarray(
            np.stack([inp['lnf_g'], inp['lnf_b']], axis=0)
            .reshape(2, 8, P).transpose(2, 0, 1).reshape(P, 16).astype(np.float32)),
    }
    # causal 0/1 masks for diagonal key-chunks (uniform across cores)
    cmask = np.zeros((4, P, TC), np.float32)
    c = np.arange(TC)[None, :]
    p = np.arange(P)[:, None]
    for j in range(4):
        cmask[j] = (c >= P * j + p).astype(np.float32)
    shared['cmask'] = cmask.astype(bf)

    # per-hf weight variants (heads hf*8 .. hf*8+8)
    Wq = np.asarray(inp['Wq'], np.float32)
    Wk = np.asarray(inp['Wk'], np.float32)
    Wv = np.asarray(inp['Wv'], np.float32)
    Wo = np.asarray(inp['Wo'], np.float32)
    hf_w = []
    for hf in range(2):
        hs = slice(hf * 8, hf * 8 + 8)
        WqT = Wq[:, hs].transpose(0, 2, 1, 3).reshape(L, D, 512)
        WkT = Wk[:, hs].transpose(0, 2, 1, 3).reshape(L, D, 512)
        WvT = Wv[:, hs].transpose(0, 2, 1, 3).reshape(L, D, 512)
        wqp = np.ascontiguousarray(
            WqT.reshape(L, 8, P, 4, P).transpose(0, 2, 1, 3, 4)).astype(bf)
        wkp = np.ascontiguousarray(
            WkT.reshape(L, 8, P, 4, P).transpose(0, 2, 1, 3, 4)).astype(bf)
        wvp = np.ascontiguousarray(
            WvT.reshape(L, 8, P, 512).transpose(0, 2, 1, 3)).astype(bf)
        wop = np.ascontiguousarray(
            Wo[:, hf * 512:(hf + 1) * 512, :]
            .reshape(L, 4, P, 8, P).transpose(0, 2, 1, 3, 4)).astype(bf)
        bo = inp['bo'] if hf == 0 else np.zeros_like(inp['bo'])
        lnpp = np.ascontiguousarray(
            np.stack([inp['ln1_g'], inp['ln1_b'], inp['ln2_g'], inp['ln2_b'],
                      bo, inp['b2']], axis=1)
            .reshape(L, 6, 8, P).transpose(0, 3, 1, 2)
            .reshape(L, P, 48).astype(np.float32))
        hf_w.append({'wqp': wqp, 'wkp': wkp, 'wvp': wvp, 'wop': wop,
                     'lnpp': lnpp})

    in_maps = []
    for cix in range(NCORES):
        b, hf = cix // 2, cix % 2
        m = dict(shared)
        m.update(hf_w[hf])
        toks = tok[b, hf * TC:(hf + 1) * TC]
        x0 = temb[toks] + pe[hf * TC:(hf + 1) * TC]      # [TC, D]
        m['x0T'] = np.ascontiguousarray(x0.T, dtype=np.float32)
        in_maps.append(m)
    return in_maps


def _enable_jax_cache():
    try:
        import jax
        jax.config.update("jax_compilation_cache_dir", "/tmp/jax_neff_cache")
        jax.config.update("jax_persistent_cache_min_compile_time_secs", 0.0)
        jax.config.update("jax_persistent_cache_min_entry_size_bytes", -1)
    except Exception:
        pass


def kernel(**inputs):
    global LAST_RESULTS
    _enable_jax_cache()
    from concourse.bass_utils import run_bass_kernel_spmd
    if 'nc' not in _CACHE:
        _CACHE['nc'] = _build()
    nc = _CACHE['nc']
    in_maps = _prep_inputs(inputs)
    kw = {}
    if TRACE:
        kw = dict(trace=True, trace_cores=list(range(NCORES)), stitch_traces=False)
    res = run_bass_kernel_spmd(nc, in_maps, core_ids=list(range(NCORES)), **kw)
    LAST_RESULTS = res
    full = np.empty((B, T, V), np.float32)
    for c in range(NCORES):
        b, hf = c // 2, c % 2
        full[b, hf * TC:(hf + 1) * TC, :] = res.results[c]['out'].T
    return full


# revision 26
# speedup vs baseline: 1.0962x; 1.0962x over previous
"""Self-contained 8-core Trainium2 kernel for the 6-layer dense transformer.

Sharding: batch pairs with head-split attention. Core c owns batch b=c//2 and
sequence half hf=c%2 (512 tokens) for the residual stream, LayerNorms, FFN and
LM head. Attention for batch b is split by heads across the pair: core 2b
computes heads 0-7, core 2b+1 heads 8-15, each over all 1024 tokens, so K/V
never cross cores. The only collectives are pairwise: an AllGather of the LN1
output (so both cores see all 1024 tokens) and a ReduceScatter of the Wo
partial products (each core receives the summed attention output for its own
tokens). The LM head is local: every core holds the full bf16 W_out.

Activations live feature-major (x^T: [D, tokens]); residual stays f32,
LN statistics f32r, everything else (projections, attention, FFN, LM head)
bf16 with f32 PSUM accumulation. Weights are pre-tiled on the host so every
weight DMA is partition-contiguous. Softmax denominators and LN statistics are
broadcast along partitions via ones-matmuls first, then inverted with a
full-partition vector reciprocal (1-partition DVE ops are ~25x slower).
The embedding lookup runs on the host; the device loads x0^T directly.
"""

import numpy as np

B, T, D, H, HS, L, DFF, V = 4, 1024, 1024, 16, 64, 6, 4096, 32000
NCORES = 8
TC = 512            # tokens owned per core
TB = 1024           # tokens per batch (attention span)
P = 128
VB = V // P         # 250 vocab row-blocks
EPS = 1e-5

_CACHE = {}
TRACE = False
LAST_RESULTS = None


def _build():
    import concourse.bacc as bacc
    import concourse.tile as tile
    import concourse.mybir as mybir
    from contextlib import ExitStack

    f32 = mybir.dt.float32
    f32r = mybir.dt.float32r
    bf16 = mybir.dt.bfloat16
    AF = mybir.ActivationFunctionType
    ALU = mybir.AluOpType

    nc = bacc.Bacc(None, target_bir_lowering=False, debug=False,
                   num_devices=NCORES)

    # ---- parameters (host pre-tiled layouts; wq/wk/wv/wo/lnpp per-core) ----
    x0T = nc.declare_dram_parameter("x0T", [D, TC], f32, isOutput=False)
    wqp = nc.declare_dram_parameter("wqp", [L, P, 8, 4, P], bf16, isOutput=False)
    wkp = nc.declare_dram_parameter("wkp", [L, P, 8, 4, P], bf16, isOutput=False)
    wvp = nc.declare_dram_parameter("wvp", [L, P, 8, 512], bf16, isOutput=False)
    wop = nc.declare_dram_parameter("wop", [L, P, 4, 8, P], bf16, isOutput=False)
    w1p = nc.declare_dram_parameter("w1p", [L, 32, P, 8, P], bf16, isOutput=False)
    w2p = nc.declare_dram_parameter("w2p", [L, 4, 8, P, 8, P], bf16, isOutput=False)
    woutp = nc.declare_dram_parameter("woutp", [VB, P, 8, P], bf16, isOutput=False)
    boutp = nc.declare_dram_parameter("boutp", [P, VB], f32, isOutput=False)
    lnpp = nc.declare_dram_parameter("lnpp", [L, P, 48], f32, isOutput=False)
    b1p = nc.declare_dram_parameter("b1p", [L, P, 32], f32, isOutput=False)
    lnfp = nc.declare_dram_parameter("lnfp", [P, 16], f32, isOutput=False)
    cmask = nc.declare_dram_parameter("cmask", [4, P, TC], bf16, isOutput=False)
    out = nc.declare_dram_parameter("out", [V, TC], f32, isOutput=True)

    RG2 = [[0, 1], [2, 3], [4, 5], [6, 7]]

    with tile.TileContext(nc) as tc:
        outer = ExitStack()
        singles = outer.enter_context(tc.tile_pool(name="singles", bufs=1))
        dramp = outer.enter_context(tc.tile_pool(name="dramp", bufs=1, space="DRAM"))

        # ---- internal DRAM (per layer and split in halves so each collective
        # can start as soon as its half of the data is staged) ----
        HD = D // 2
        xln_locs = [[dramp.tile([HD, TC], bf16, name=f"xln_loc_{i}_{h}")
                     for h in range(2)] for i in range(L)]
        xln_pairs = [[dramp.tile([D, TC], bf16, name=f"xln_pair_{i}_{h}")
                      for h in range(2)] for i in range(L)]
        part_locs = [[dramp.tile([D, TC], bf16, name=f"part_loc_{i}_{h}")
                      for h in range(2)] for i in range(L)]
        attn_rss = [[dramp.tile([HD, TC], bf16, name=f"attn_rs_{i}_{h}")
                     for h in range(2)] for i in range(L)]

        # constants
        ones_f = singles.tile([P, 144], f32, name="ones_f")
        nc.vector.memset(ones_f[:], 1.0)
        ones_r = singles.tile([P, 144], f32r, name="ones_r")
        nc.vector.tensor_copy(out=ones_r[:], in_=ones_f[:])
        eps_c = singles.tile([P, 1], f32, name="eps_c")
        nc.vector.memset(eps_c[:], EPS)
        ones_b = singles.tile([P, P], bf16, name="ones_b")
        nc.vector.memset(ones_b[:], 1.0)
        bout_t = singles.tile([P, VB], f32, name="bout_t")
        nc.sync.dma_start(out=bout_t[:], in_=boutp[:])
        cm_t = []
        for j in range(4):
            mt = singles.tile([P, TC], bf16, name=f"cm_{j}")
            nc.sync.dma_start(out=mt[:], in_=cmask[j])
            cm_t.append(mt)

        est = ExitStack()
        lp = est.enter_context(tc.tile_pool(name="lp", bufs=1))      # xr/xln tiles
        kqp = est.enter_context(tc.tile_pool(name="kqp", bufs=1))    # K/Q [P,1024]
        vtsp = est.enter_context(tc.tile_pool(name="vtsp", bufs=1))  # V 65-strided
        osp = est.enter_context(tc.tile_pool(name="osp", bufs=1))    # o [P,1024]
        big = est.enter_context(tc.tile_pool(name="big", bufs=1))    # h1/y2 slots
        wL = est.enter_context(tc.tile_pool(name="wL", bufs=1))      # layer weights
        wA = est.enter_context(tc.tile_pool(name="wA", bufs=3))      # ffn stream
        sp = est.enter_context(tc.tile_pool(name="sp", bufs=2))      # stream tiles
        esp = est.enter_context(tc.tile_pool(name="esp", bufs=3))    # exp(scores)
        otp = est.enter_context(tc.tile_pool(name="otp", bufs=2))    # denom tmp
        stp = est.enter_context(tc.tile_pool(name="stp", bufs=1))    # LN stats [1,*]

        ps_mm = est.enter_context(tc.tile_pool(name="ps_mm", bufs=2, space="PSUM"))
        ps_o = est.enter_context(tc.tile_pool(name="ps_o", bufs=1, space="PSUM"))
        ps_st = est.enter_context(tc.tile_pool(name="ps_st", bufs=1, space="PSUM"))
        ps_bc = est.enter_context(tc.tile_pool(name="ps_bc", bufs=1, space="PSUM"))

        def mmtile():
            return ps_mm.tile([P, TC], f32, name="mm", tag="mm")

        xr = [lp.tile([P, TC], f32, name=f"xr_{j}", tag=f"xr_{j}") for j in range(8)]

        def own_tiles(dtype=bf16):
            return [lp.tile([P, TC], dtype, name=f"xln_{j}", tag=f"xln_{j}")
                    for j in range(8)]

        # persistent attention tiles. kz: one tile per head, the head's 64 K
        # rows in its parity half and ZEROS in the other half, so the score
        # matmul runs with a full 128x128 stationary (full PE rate); the rhs
        # reads the packed Q tile whose other half contributes 0 via the zeros.
        kz = [kqp.tile([P, TB], bf16, name=f"kz_{i}", tag=f"kz_{i}")
              for i in range(8)]
        for i in range(8):
            nc.vector.memset(kz[i][:], 0.0)
        kq_q = [kqp.tile([P, TB], bf16, name=f"kq_{i}", tag=f"kq_{i}")
                for i in range(4)]
        # 584 wide so a 128-col stationary window starting at 65*hh always fits
        vts = [vtsp.tile([P, 584], bf16, name=f"vts_{i}", tag=f"vts_{i}")
               for i in range(8)]
        for i in range(8):
            nc.vector.memset(vts[i][:], 1.0)   # ones cols (64 of each 65) persist
        o_sb = [osp.tile([P, TB], bf16, name=f"osb_{i}", tag=f"osb_{i}")
                for i in range(4)]

        def big_tile(i, name, dtype=bf16):
            return big.tile([P, TC], dtype, name=name, tag=f"big_{i}")

        # ---- embedding: host-precomputed x0T, straight loads ----
        for j in range(8):
            nc.sync.dma_start(out=xr[j][:], in_=x0T[j * P:(j + 1) * P, :])

        def layer_norm(g_t, gcol, b_t, bcol, out_tiles):
            """xr (f32) -> out_tiles; feature-major LN over partitions.

            Stats accumulate through a full 128x128 ones stationary, which
            lands them already broadcast along partitions at full PE rate.
            """
            sum_bc = ps_bc.tile([P, TC], f32, name="sum_bc", tag="bc_a")
            sumsq_bc = ps_bc.tile([P, TC], f32, name="sumsq_bc", tag="bc_c")
            for j in range(8):
                xc = sp.tile([P, TC], bf16, name="ln_xc", tag="ln_xc")
                nc.scalar.activation(out=xc[:], in_=xr[j][:], func=AF.Copy)
                sq = sp.tile([P, TC], bf16, name="ln_sq", tag="ln_sq")
                nc.scalar.activation(out=sq[:], in_=xr[j][:], func=AF.Square)
                nc.tensor.matmul(out=sum_bc[:], lhsT=ones_b[:], rhs=xc[:],
                                 start=(j == 0), stop=(j == 7))
                nc.tensor.matmul(out=sumsq_bc[:], lhsT=ones_b[:], rhs=sq[:],
                                 start=(j == 0), stop=(j == 7))
            nmean = sp.tile([P, TC], f32, name="ln_mb", tag="ln_mb")
            nc.scalar.activation(out=nmean[:], in_=sum_bc[:], func=AF.Copy,
                                 scale=-1.0 / D)
            msq = sp.tile([P, TC], f32, name="ln_msq", tag="ln_msq")
            nc.scalar.activation(out=msq[:], in_=sumsq_bc[:], func=AF.Copy,
                                 scale=1.0 / D)
            m2 = sp.tile([P, TC], f32, name="ln_m2b", tag="ln_m2b")
            nc.vector.tensor_mul(out=m2[:], in0=nmean[:], in1=nmean[:])
            nc.vector.tensor_tensor(out=msq[:], in0=msq[:], in1=m2[:],
                                    op=ALU.subtract)
            std = sp.tile([P, TC], f32, name="ln_sb", tag="ln_sb")
            nc.scalar.activation(out=std[:], in_=msq[:], func=AF.Sqrt,
                                 bias=eps_c[:], scale=1.0)
            rstd_bc = sp.tile([P, TC], f32, name="ln_rb", tag="ln_rb")
            nc.vector.reciprocal(out=rstd_bc[:], in_=std[:])
            for j in range(8):
                t1 = sp.tile([P, TC], f32, name="ln_t1", tag="ln_t1")
                nc.vector.tensor_add(out=t1[:], in0=xr[j][:], in1=nmean[:])
                nc.vector.tensor_mul(out=t1[:], in0=t1[:], in1=rstd_bc[:])
                nc.vector.tensor_scalar(
                    out=out_tiles[j][:], in0=t1[:],
                    scalar1=g_t[:, gcol + j:gcol + j + 1],
                    scalar2=b_t[:, bcol + j:bcol + j + 1],
                    op0=ALU.mult, op1=ALU.add)

        # ================= layers =================
        for l in range(L):
            lnp = sp.tile([P, 48], f32, name="lnp", tag="lnp")
            nc.sync.dma_start(out=lnp[:], in_=lnpp[l])
            b1f_t = sp.tile([P, 32], f32, name="b1f_t", tag="b1f_t")
            nc.sync.dma_start(out=b1f_t[:], in_=b1p[l])

            # ---- LN1 on own tokens -> store -> two pipelined pair AllGathers ----
            xln1 = own_tiles()
            layer_norm(lnp, 0, lnp, 8, xln1)
            for h in range(2):
                for j4 in range(4):
                    nc.sync.dma_start(
                        out=xln_locs[l][h][j4 * P:(j4 + 1) * P, :],
                        in_=xln1[4 * h + j4][:])
                nc.gpsimd.collective_compute(
                    "AllGather", ALU.bypass, replica_groups=RG2,
                    ins=[xln_locs[l][h].opt()], outs=[xln_pairs[l][h].opt()])

            # reload the gathered 1024-token activations (feature-major)
            xf_full = [lp.tile([P, TB], bf16, name=f"xf_{j}", tag=f"xf_{j}")
                       for j in range(8)]
            for k in range(8):
                h, k4 = k // 4, k % 4
                nc.sync.dma_start(out=xf_full[k][:, 0:TC],
                                  in_=xln_pairs[l][h][k4 * P:(k4 + 1) * P, :])
                nc.sync.dma_start(
                    out=xf_full[k][:, TC:TB],
                    in_=xln_pairs[l][h][HD + k4 * P:HD + (k4 + 1) * P, :])

            # ---- layer weight tiles (one contiguous DMA each) ----
            wk_t = wL.tile([P, 8, 4, P], bf16, name="wk_t", tag="wk")
            nc.sync.dma_start(out=wk_t[:], in_=wkp[l])
            wq_t = wL.tile([P, 8, 4, P], bf16, name="wq_t", tag="wq")
            nc.sync.dma_start(out=wq_t[:], in_=wqp[l])
            wv_t = wL.tile([P, 8, 512], bf16, name="wv_t", tag="wv")
            nc.sync.dma_start(out=wv_t[:], in_=wvp[l])
            wo_t = wL.tile([P, 4, 8, P], bf16, name="wo_t", tag="wo")
            nc.sync.dma_start(out=wo_t[:], in_=wop[l])

            # ---- K and Q projections: my 8 heads x 1024 tokens ----
            for oc in range(4):
                for th in range(2):
                    ps = mmtile()
                    for k in range(8):
                        nc.tensor.matmul(
                            out=ps[:], lhsT=wk_t[:, k, oc, :],
                            rhs=xf_full[k][:, th * TC:(th + 1) * TC],
                            start=(k == 0), stop=(k == 7))
                    # split row halves into the two heads' zero-padded K tiles
                    for ph in range(2):
                        nc.scalar.activation(
                            out=kz[2 * oc + ph][64 * ph:64 * ph + 64,
                                                th * TC:(th + 1) * TC],
                            in_=ps[64 * ph:64 * ph + 64, :], func=AF.Copy)
            for oc in range(4):
                for th in range(2):
                    ps = mmtile()
                    for k in range(8):
                        nc.tensor.matmul(
                            out=ps[:], lhsT=wq_t[:, k, oc, :],
                            rhs=xf_full[k][:, th * TC:(th + 1) * TC],
                            start=(k == 0), stop=(k == 7))
                    nc.scalar.activation(
                        out=kq_q[oc][:, th * TC:(th + 1) * TC], in_=ps[:],
                        func=AF.Copy)

            # ---- V projection: [tokens, head-dims], 65-strided with ones ----
            for tcn in range(8):
                ps = mmtile()
                for k in range(8):
                    nc.tensor.matmul(
                        out=ps[:], lhsT=xf_full[k][:, tcn * P:(tcn + 1) * P],
                        rhs=wv_t[:, k, :], start=(k == 0), stop=(k == 7))
                vsb = sp.tile([P, 512], bf16, name="vsb", tag="vsb")
                nc.scalar.activation(out=vsb[:], in_=ps[:], func=AF.Copy)
                nc.sync.dma_start(
                    out=vts[tcn][:, 0:520].rearrange("p (h c) -> p h c",
                                                     c=65)[:, :, 0:64],
                    in_=vsb[:].rearrange("p (h c) -> p h c", c=64))

            # ---- attention: 8 local heads, interleaved in pairs so one head's
            # matmuls fill the other's softmax bubbles ----
            for hp in range(4):
                qt = kq_q[hp]
                opst = [None, None]
                for sl in range(2):       # slot: even head -> ps_o, odd -> ps_st
                    pso = ps_o if sl == 0 else ps_st
                    tga, tgb = (("ops_lo", "ops_hi") if sl == 0
                                else ("st_a", "st_b"))
                    opst[sl] = (pso.tile([P, TC], f32, name="ops_lo", tag=tga),
                                pso.tile([P, TC], f32, name="ops_hi", tag=tgb))
                for j in range(8):
                    for sl in range(2):
                        hh = 2 * hp + sl
                        ops_lo, ops_hi = opst[sl]
                        vwin = vts[j][:, 65 * hh:65 * hh + P]
                        es_hi = esp.tile([P, TC], bf16, name="es_h",
                                         tag=f"es_h{sl}")
                        if j < 4:
                            # lo half: q columns [128j, 512), diagonal-masked
                            q0 = P * j
                            sps = mmtile()
                            nc.tensor.matmul(
                                out=sps[:, q0:TC],
                                lhsT=kz[hh][:, j * P:(j + 1) * P],
                                rhs=qt[:, q0:TC],
                                start=True, stop=True)
                            es_lo = esp.tile([P, TC], bf16, name="es_l",
                                             tag=f"es_l{sl}")
                            if j > 0:
                                nc.vector.memset(es_lo[:, 0:q0], 0.0)
                            nc.scalar.activation(out=es_lo[:, q0:TC],
                                                 in_=sps[:, q0:TC],
                                                 func=AF.Exp, scale=HS ** -0.5)
                            nc.vector.tensor_mul(out=es_lo[:, q0:TC],
                                                 in0=es_lo[:, q0:TC],
                                                 in1=cm_t[j][:, q0:TC])
                            nc.tensor.matmul(
                                out=ops_lo[:], lhsT=vwin,
                                rhs=es_lo[:], start=(j == 0), stop=(j == 3))
                            # hi half: q columns [512, 1024), fully visible
                            sps2 = mmtile()
                            nc.tensor.matmul(
                                out=sps2[:],
                                lhsT=kz[hh][:, j * P:(j + 1) * P],
                                rhs=qt[:, TC:TB],
                                start=True, stop=True)
                            nc.scalar.activation(out=es_hi[:], in_=sps2[:],
                                                 func=AF.Exp, scale=HS ** -0.5)
                        else:
                            # hi half only: q columns [512+128(j-4), 1024)
                            q0 = P * (j - 4)
                            sps = mmtile()
                            nc.tensor.matmul(
                                out=sps[:, q0:TC],
                                lhsT=kz[hh][:, j * P:(j + 1) * P],
                                rhs=qt[:, TC + q0:TB],
                                start=True, stop=True)
                            if j > 4:
                                nc.vector.memset(es_hi[:, 0:q0], 0.0)
                            nc.scalar.activation(out=es_hi[:, q0:TC],
                                                 in_=sps[:, q0:TC],
                                                 func=AF.Exp, scale=HS ** -0.5)
                            nc.vector.tensor_mul(out=es_hi[:, q0:TC],
                                                 in0=es_hi[:, q0:TC],
                                                 in1=cm_t[j - 4][:, q0:TC])
                        nc.tensor.matmul(
                            out=ops_hi[:], lhsT=vwin,
                            rhs=es_hi[:], start=(j == 0), stop=(j == 7))
                # normalize: o = ops[0:64] / ops[64] (ones row)
                for sl in range(2):
                    hh = 2 * hp + sl
                    base = 64 * sl
                    for ih, opsx in ((0, opst[sl][0]), (1, opst[sl][1])):
                        dsb = otp.tile([1, TC], f32r, name="dsb", tag="dsb")
                        with nc.allow_low_precision(reason="f32r softmax denom"):
                            nc.scalar.activation(out=dsb[:], in_=opsx[64:65, :],
                                                 func=AF.Copy)
                        bc = ps_bc.tile([P, TC], f32, name="bc_o", tag="bc_a")
                        nc.tensor.matmul(out=bc[:], lhsT=ones_r[0:1, 0:P],
                                         rhs=dsb[:], start=True, stop=True)
                        bcr = otp.tile([64, TC], bf16, name="bcr", tag="bcr")
                        with nc.allow_low_precision(reason="bf16 denom recip"):
                            nc.vector.reciprocal(out=bcr[:], in_=bc[0:64, :])
                        nc.vector.tensor_mul(
                            out=o_sb[hp][base:base + 64, ih * TC:(ih + 1) * TC],
                            in0=opsx[0:64, :], in1=bcr[:])

            # ---- Wo partials -> staging -> two pipelined pair ReduceScatters ----
            for h in range(2):
                for d4 in range(4):
                    dout = 4 * h + d4
                    for th in range(2):
                        ps = mmtile()
                        for kc in range(4):
                            nc.tensor.matmul(
                                out=ps[:], lhsT=wo_t[:, kc, dout, :],
                                rhs=o_sb[kc][:, th * TC:(th + 1) * TC],
                                start=(kc == 0), stop=(kc == 3))
                        psb = sp.tile([P, TC], bf16, name="psb", tag="psb")
                        nc.scalar.activation(out=psb[:], in_=ps[:],
                                             func=AF.Identity,
                                             bias=lnp[:, 32 + dout:33 + dout],
                                             scale=1.0)
                        nc.sync.dma_start(
                            out=part_locs[l][h][th * HD + d4 * P:
                                                th * HD + (d4 + 1) * P, :],
                            in_=psb[:])
                nc.gpsimd.collective_compute(
                    "ReduceScatter", ALU.add, replica_groups=RG2,
                    ins=[part_locs[l][h].opt()], outs=[attn_rss[l][h].opt()])

            # ---- residual add from the scattered attention output ----
            for dout in range(8):
                h, d4 = dout // 4, dout % 4
                ar = sp.tile([P, TC], bf16, name="ar", tag="ar")
                nc.sync.dma_start(out=ar[:],
                                  in_=attn_rss[l][h][d4 * P:(d4 + 1) * P, :])
                nc.vector.tensor_add(out=xr[dout][:], in0=xr[dout][:], in1=ar[:])

            # ---- FFN (own 512 tokens, bf16) ----
            xln2 = own_tiles()
            layer_norm(lnp, 16, lnp, 24, xln2)

            y2 = [big_tile(8 + d_, f"y2_{d_}", dtype=f32) for d_ in range(8)]
            for blk in range(4):
                h1 = [big_tile(c_, f"h1_{c_}") for c_ in range(8)]
                for ci in range(8):
                    hc = 8 * blk + ci
                    wt = wA.tile([P, 8, P], bf16, name="w1_t", tag="wA")
                    nc.sync.dma_start(out=wt[:], in_=w1p[l, hc])
                    ps = mmtile()
                    for k in range(8):
                        nc.tensor.matmul(out=ps[:], lhsT=wt[:, k, :], rhs=xln2[k][:],
                                         start=(k == 0), stop=(k == 7))
                    nc.scalar.activation(out=h1[ci][:], in_=ps[:], func=AF.Relu,
                                         bias=b1f_t[:, hc:hc + 1], scale=1.0)
                for dout in range(8):
                    wt = wA.tile([P, 8, P], bf16, name="w2_t", tag="wA")
                    nc.sync.dma_start(out=wt[:], in_=w2p[l, blk, dout])
                    ps = mmtile()
                    for c in range(8):
                        nc.tensor.matmul(out=ps[:], lhsT=wt[:, c, :], rhs=h1[c][:],
                                         start=(c == 0), stop=(c == 7))
                    if blk == 0:
                        nc.scalar.activation(out=y2[dout][:], in_=ps[:],
                                             func=AF.Identity,
                                             bias=lnp[:, 40 + dout:41 + dout],
                                             scale=1.0)
                    else:
                        nc.vector.tensor_add(out=y2[dout][:], in0=y2[dout][:],
                                             in1=ps[:])
            for dout in range(8):
                nc.vector.tensor_add(out=xr[dout][:], in0=xr[dout][:],
                                     in1=y2[dout][:])

        # ---- final LN (bf16 output for the LM head) ----
        lnf_t = sp.tile([P, 16], f32, name="lnf_t", tag="lnp")
        nc.sync.dma_start(out=lnf_t[:], in_=lnfp[:])
        xlnf = own_tiles()
        layer_norm(lnf_t, 0, lnf_t, 8, xlnf)

        est.close()

        # ================= LM head (local, full vocab) =================
        with tc.tile_pool(name="lmw", bufs=3) as lmw, \
             tc.tile_pool(name="lmo", bufs=3) as lmo, \
             tc.tile_pool(name="ps_lm", bufs=4, space="PSUM") as ps_lm:
            for vb in range(VB):
                wt = lmw.tile([P, 8, P], bf16, name="wout_t", tag="wout")
                nc.sync.dma_start(out=wt[:], in_=woutp[vb])
                ps = ps_lm.tile([P, TC], f32, name="lm_ps", tag="lm")
                for k in range(8):
                    nc.tensor.matmul(out=ps[:], lhsT=wt[:, k, :], rhs=xlnf[k][:],
                                     start=(k == 0), stop=(k == 7))
                osb = lmo.tile([P, TC], f32, name="osb", tag="osb")
                nc.scalar.activation(out=osb[:], in_=ps[:], func=AF.Identity,
                                     bias=bout_t[:, vb:vb + 1], scale=1.0)
                nc.sync.dma_start(out=out[vb * P:(vb + 1) * P, :], in_=osb[:])
        outer.close()

    nc.compile()
    return nc


def _prep_inputs(inputs):
    """Shard/reformat host inputs into 8 per-core input maps."""
    import ml_dtypes
    bf = ml_dtypes.bfloat16
    inp = {k: np.asarray(v) for k, v in inputs.items()}
    tok = inp['input_tokens'].astype(np.int64)          # [B, T]
    temb = np.asarray(inp['tok_emb'], dtype=np.float32)
    pe = np.asarray(inp['pos_emb'], dtype=np.float32)

    w1 = np.asarray(inp['W1'], np.float32)
    w2 = np.asarray(inp['W2'], np.float32)
    shared = {
        'w1p': np.ascontiguousarray(
            w1.reshape(L, 8, P, 32, P).transpose(0, 3, 2, 1, 4)).astype(bf),
        'w2p': np.ascontiguousarray(
            w2.reshape(L, 4, 8, P, 8, P).transpose(0, 1, 4, 3, 2, 5)).astype(bf),
        'woutp': np.ascontiguousarray(
            np.asarray(inp['W_out'], np.float32)
            .reshape(8, P, VB, P).transpose(2, 1, 0, 3)).astype(bf),
        'boutp': np.ascontiguousarray(
            np.asarray(inp['b_out'], np.float32).reshape(VB, P).T),
        'b1p': np.ascontiguousarray(
            np.asarray(inp['b1'], np.float32).reshape(L, 32, P).transpose(0, 2, 1)),
        'lnfp': np.ascontiguousarray(
            np.stack([inp['lnf_g'], inp['lnf_b']], axis=0)
            .reshape(2, 8, P).transpose(2, 0, 1).reshape(P, 16).astype(np.float32)),
    }
    # causal 0/1 masks for diagonal key-chunks (uniform across cores)
    cmask = np.zeros((4, P, TC), np.float32)
    c = np.arange(TC)[None, :]
    p = np.arange(P)[:, None]
    for j in range(4):
        cmask[j] = (c >= P * j + p).astype(np.float32)
    shared['cmask'] = cmask.astype(bf)

    # per-hf weight variants (heads hf*8 .. hf*8+8)
    Wq = np.asarray(inp['Wq'], np.float32)
    Wk = np.asarray(inp['Wk'], np.float32)
    Wv = np.asarray(inp['Wv'], np.float32)
    Wo = np.asarray(inp['Wo'], np.float32)
    hf_w = []
    for hf in range(2):
        hs = slice(hf * 8, hf * 8 + 8)
        WqT = Wq[:, hs].transpose(0, 2, 1, 3).reshape(L, D, 512)
        WkT = Wk[:, hs].transpose(0, 2, 1, 3).reshape(L, D, 512)
        WvT = Wv[:, hs].transpose(0, 2, 1, 3).reshape(L, D, 512)
        wqp = np.ascontiguousarray(
            WqT.reshape(L, 8, P, 4, P).transpose(0, 2, 1, 3, 4)).astype(bf)
        wkp = np.ascontiguousarray(
            WkT.reshape(L, 8, P, 4, P).transpose(0, 2, 1, 3, 4)).astype(bf)
        wvp = np.ascontiguousarray(
            WvT.reshape(L, 8, P, 512).transpose(0, 2, 1, 3)).astype(bf)
        wop = np.ascontiguousarray(
            Wo[:, hf * 512:(hf + 1) * 512, :]
            .reshape(L, 4, P, 8, P).transpose(0, 2, 1, 3, 4)).astype(bf)
        bo = inp['bo'] if hf == 0 else np.zeros_like(inp['bo'])
        lnpp = np.ascontiguousarray(
            np.stack([inp['ln1_g'], inp['ln1_b'], inp['ln2_g'], inp['ln2_b'],
                      bo, inp['b2']], axis=1)
            .reshape(L, 6, 8, P).transpose(0, 3, 1, 2)
            .reshape(L, P, 48).astype(np.float32))
        hf_w.append({'wqp': wqp, 'wkp': wkp, 'wvp': wvp, 'wop': wop,
                     'lnpp': lnpp})

    in_maps = []
    for cix in range(NCORES):
        b, hf = cix // 2, cix % 2
        m = dict(shared)
        m.update(hf_w[hf])
        toks = tok[b, hf * TC:(hf + 1) * TC]
        x0 = temb[toks] + pe[hf * TC:(hf + 1) * TC]      # [TC, D]
        m['x0T'] = np.ascontiguousarray(x0.T, dtype=np.float32)
        in_maps.append(m)
    return in_maps


def _enable_jax_cache():
    try:
        import jax
        jax.config.update("jax_compilation_cache_dir", "/tmp/jax_neff_cache")
        jax.config.update("jax_persistent_cache_min_compile_time_secs", 0.0)
        jax.config.update("jax_persistent_cache_min_entry_size_bytes", -1)
    except Exception:
        pass


def kernel(**inputs):
    global LAST_RESULTS
    _enable_jax_cache()
    from concourse.bass_utils import run_bass_kernel_spmd
    if 'nc' not in _CACHE:
        _CACHE['nc'] = _build()
    nc = _CACHE['nc']
    in_maps = _prep_inputs(inputs)
    kw = {}
    if TRACE:
        kw = dict(trace=True, trace_cores=list(range(NCORES)), stitch_traces=False)
    res = run_bass_kernel_spmd(nc, in_maps, core_ids=list(range(NCORES)), **kw)
    LAST_RESULTS = res
    full = np.empty((B, T, V), np.float32)
    for c in range(NCORES):
        b, hf = c // 2, c % 2
        full[b, hf * TC:(hf + 1) * TC, :] = res.results[c]['out'].T
    return full
